# revision 1
# baseline (speedup 1.0000x reference)
"""Trainium2 Bass kernel for nn_DistanceModel1 (quantum-embedding trace
distance model).

Math: psi_b = exp(-0.5j*phase_b)/16 with theta = 0.5*phase, so with
C = cos(theta), S = sin(theta) in [B, 256]:
  256*B*Re(rho) = C^T C + S^T S
  256*B*Im(rho) = C^T S - (C^T S)^T
The answer -0.5*sum|eig(rho1 - rho0)| is the trace norm of the Hermitian
difference, computed with a matrix-sign (polar) iteration: sum|lam| =
tr(sign(A) * A), sign via a tuned odd-quintic schedule + one final cubic.

Distribution: data-parallel over batch on 8 NeuronCores, one AllReduce of
the two 256x256 Gram-difference matrices, then a replicated eigensolve.

All matmul operands are stored as bf16 (explicit RNE rounding on the
producing engine); PSUM accumulation is fp32 throughout.
"""

import numpy as np
import ml_dtypes

import concourse.bass as bass
import concourse.mybir as mybir
import concourse.tile as tile
from concourse import bacc
from concourse.bass_utils import run_bass_kernel_spmd

F32 = mybir.dt.float32
BF16 = mybir.dt.bfloat16

N_CORES = 8
B_TOT = 65536
B_LOC = B_TOT // N_CORES          # 8192 per side per core
BL2 = 2 * B_LOC                   # 16384: [x1-shard | x0-shard]
DIM = 256
N_CHUNK = BL2 // 512              # 32 MLP chunks of 512
N_PACK = BL2 // 256               # 64 gram packs of 256 rows (2x128)
PI = float(np.pi)

S_SCALE = 0.0075                  # spectral normalization |lam|max ~ 0.0065
ALPHA = 1.0 / (256.0 * B_TOT * S_SCALE)

# tuned odd-quintic sign-iteration schedule (see tune.py); applied as
# x <- a x + b x^3 + c x^5, followed by one Newton-Schulz cubic.
SCHED = [
    (5.5291767399140692, -16.389957534164846, 12.160780794250778),
    (4.2403211966366081, -7.3962303756276766, 3.2419284578310239),
    (4.1855655241270746, -7.1285927095774353, 3.1027462404570993),
    (3.955629702304988, -6.0759597846319524, 2.5603676614256519),
    (3.1616509709539757, -3.2426616827825416, 1.1618772184765096),
    (1.621445550205223, -0.7986253407700471, 0.17471394782073113),
]
CUBIC = (1.5, -0.5)


def _rb(a):
    return np.asarray(a, dtype=ml_dtypes.bfloat16)


def _build_ghat():
    """Ghat [16, 256] fp32: theta = v @ Ghat with v = [h(8), p(7), 1],
    p_j = h_j*h_{j+1} (the zz term expanded so only the bilinear part
    needs on-device compute)."""
    n = 8
    d = 256
    bits = (np.arange(d)[:, None] >> (n - 1 - np.arange(n))[None, :]) & 1
    signs = (1.0 - 2.0 * bits).astype(np.float64)           # [256, 8]
    pair = signs[:, :-1] * signs[:, 1:]                      # [256, 7]
    G = np.zeros((16, d), dtype=np.float64)
    for f in range(8):
        col = signs[:, f].copy()
        if f >= 1:
            col += -PI * pair[:, f - 1]
        if f <= 6:
            col += -PI * pair[:, f]
        G[f] = 0.5 * col
    for j in range(7):
        G[8 + j] = 0.5 * pair[:, j]
    G[15] = 0.5 * PI * PI * pair.sum(axis=1)
    return G.astype(np.float32)


def _build_nc():
    AF = mybir.ActivationFunctionType
    OP = mybir.AluOpType

    nc = bacc.Bacc(
        "TRN2",
        target_bir_lowering=False,
        debug=False,
        enable_asserts=False,
        num_devices=N_CORES,
    )

    xs_d = nc.dram_tensor("xs", [8, BL2], BF16, kind="ExternalInput")
    w1_d = nc.dram_tensor("w1", [8, 10], BF16, kind="ExternalInput")
    w2_d = nc.dram_tensor("w2", [10, 10], BF16, kind="ExternalInput")
    w3_d = nc.dram_tensor("w3", [10, 8], BF16, kind="ExternalInput")
    bias_d = nc.dram_tensor("biases", [10, 3], F32, kind="ExternalInput")
    out_d = nc.dram_tensor("out", [1, 1], F32, kind="ExternalOutput")
    dbg_d = nc.dram_tensor("dbg", [2, 512, 256], F32, kind="ExternalOutput")
    dbg2_d = nc.dram_tensor("dbg2", [6, 128, 256], F32, kind="ExternalOutput")

    gh_d = nc.inline_tensor(_rb(_build_ghat()), "ghat")          # [16, 256]
    ones_d = nc.inline_tensor(np.ones((1, BL2), ml_dtypes.bfloat16), "onesrow")
    ident_d = nc.inline_tensor(np.eye(128, dtype=np.float32), "ident")

    with tile.TileContext(nc) as tc:
        _body(nc, tc, AF, OP, xs_d, w1_d, w2_d, w3_d, bias_d, gh_d, ones_d,
              ident_d, out_d, dbg_d, dbg2_d)
    nc.compile()
    return nc


def _body(nc, tc, AF, OP, xs_d, w1_d, w2_d, w3_d, bias_d, gh_d, ones_d,
          ident_d, out_d, dbg_d, dbg2_d=None):
    from contextlib import ExitStack
    es = ExitStack()

    constp = es.enter_context(tc.tile_pool(name="constp", bufs=1))

    xs = constp.tile([8, BL2], BF16)
    nc.sync.dma_start(out=xs, in_=xs_d[:])
    w1 = constp.tile([8, 10], BF16)
    nc.sync.dma_start(out=w1, in_=w1_d[:])
    w2 = constp.tile([10, 10], BF16)
    nc.sync.dma_start(out=w2, in_=w2_d[:])
    w3 = constp.tile([10, 8], BF16)
    nc.sync.dma_start(out=w3, in_=w3_d[:])
    biases = constp.tile([10, 3], F32)
    nc.sync.dma_start(out=biases, in_=bias_d[:])
    gh = constp.tile([16, 256], BF16)
    nc.sync.dma_start(out=gh, in_=gh_d[:])
    ident = constp.tile([128, 128], F32)
    nc.sync.dma_start(out=ident, in_=ident_d[:])
    ones_col = constp.tile([128, 1], F32)
    nc.vector.memset(ones_col, 1.0)
    zero_b = constp.tile([128, 1], F32)
    nc.vector.memset(zero_b, 0.0)

    v = constp.tile([16, BL2], BF16)       # [h(0:8); p(8:15); ones(15)]
    nc.sync.dma_start(out=v[15:16, :], in_=ones_d[:])

    # ---------------- MLP + feature build ----------------
    es_ps1 = ExitStack()
    mlp_ps = es_ps1.enter_context(tc.tile_pool(name="mlp_ps", bufs=2, space="PSUM"))
    actp = es.enter_context(tc.tile_pool(name="actp", bufs=5))

    for n in range(N_CHUNK):
        sl = slice(n * 512, (n + 1) * 512)
        pmm = mlp_ps.tile([10, 512], F32, tag="mp", name="mp")
        nc.tensor.matmul(pmm[0:10, :], lhsT=w1, rhs=xs[:, sl],
                         start=True, stop=True)
        h1c = actp.tile([10, 512], BF16, tag="h1c", name="h1c")
        nc.vector.tensor_scalar(h1c, pmm[0:10, :], biases[:, 0:1], 0.0,
                                op0=OP.add, op1=OP.max)
        pmm2 = mlp_ps.tile([10, 512], F32, tag="mp", name="mp")
        nc.tensor.matmul(pmm2[0:10, :], lhsT=w2, rhs=h1c,
                         start=True, stop=True)
        h2c = actp.tile([10, 512], BF16, tag="h2c", name="h2c")
        nc.scalar.activation(h2c, pmm2[0:10, :], AF.Relu, bias=biases[:, 1:2])
        pmm3 = mlp_ps.tile([10, 512], F32, tag="mp", name="mp")
        nc.tensor.matmul(pmm3[0:8, :], lhsT=w3, rhs=h2c,
                         start=True, stop=True)
        # h -> v[0:8] (base-0 everywhere)
        nc.vector.tensor_scalar(v[0:8, sl], pmm3[0:8, :], biases[0:8, 2:3],
                                None, op0=OP.add)
        # shifted h via DMA (partition move), then p = h_j * h_{j+1}
        hsc = actp.tile([7, 512], BF16, tag="hsc", name="hsc")
        nc.sync.dma_start(out=hsc, in_=v[1:8, sl])
        pc = actp.tile([7, 512], BF16, tag="pc", name="pc")
        nc.vector.tensor_tensor(pc, v[0:7, sl], hsc, op=OP.mult)
        nc.sync.dma_start(out=v[8:15, sl], in_=pc)

    # ---------------- Gram accumulation ----------------
    gram_ps = es_ps1.enter_context(tc.tile_pool(name="gram_ps", bufs=1, space="PSUM"))
    th_ps = es_ps1.enter_context(tc.tile_pool(name="th_ps", bufs=2, space="PSUM"))
    csp = es.enter_context(tc.tile_pool(name="csp", bufs=4))
    wrapp = es.enter_context(tc.tile_pool(name="wrapp", bufs=2))

    redp = es.enter_context(tc.tile_pool(name="redp", bufs=1))
    dramp = es.enter_context(tc.tile_pool(name="dramp", bufs=1, space="DRAM"))
    cc_in = [dramp.tile([512, 256], F32, name=f"cc_in{h}") for h in (0, 1)]
    cc_out = [dramp.tile([512, 256], F32, addr_space="Shared", name=f"cc_out{h}")
              for h in (0, 1)]

    # accumulator banks: [G1_m | G0_m], [D1_m | D0_m] as [128, 512] each
    bankG = [gram_ps.tile([128, 512], F32, tag=f"bg{m}", name=f"bg{m}") for m in (0, 1)]
    bankD = [gram_ps.tile([128, 512], F32, tag=f"bd{m}", name=f"bd{m}") for m in (0, 1)]

    def emit_epilogue(h):
        """extract Gd/Dd = side0 - side1 for batch-half h, DMA to cc_in[h],
        and kick its AllReduce (half 0 overlaps with half-1 compute)."""
        for m in (0, 1):
            t1 = redp.tile([128, 256], F32, tag=f"cp{m}{h}", name=f"cp{m}{h}")
            nc.scalar.activation(t1, bankG[m][:, 0:256], AF.Copy)
            gd = redp.tile([128, 256], F32, tag=f"gd{m}{h}", name=f"gd{m}{h}")
            nc.vector.tensor_tensor(gd, t1, bankG[m][:, 256:512], op=OP.subtract)
            nc.sync.dma_start(out=cc_in[h][m * 128:(m + 1) * 128, :], in_=gd)
            t2 = redp.tile([128, 256], F32, tag=f"cq{m}{h}", name=f"cq{m}{h}")
            nc.scalar.activation(t2, bankD[m][:, 0:256], AF.Copy)
            dd = redp.tile([128, 256], F32, tag=f"dd{m}{h}", name=f"dd{m}{h}")
            nc.vector.tensor_tensor(dd, t2, bankD[m][:, 256:512], op=OP.subtract)
            nc.sync.dma_start(out=cc_in[h][256 + m * 128:256 + (m + 1) * 128, :],
                              in_=dd)
        nc.gpsimd.collective_compute(
            "AllReduce",
            mybir.AluOpType.add,
            replica_groups=[list(range(N_CORES))],
            ins=[cc_in[h].opt()],
            outs=[cc_out[h].opt()],
        )

    for p in range(N_PACK):
        th = th_ps.tile([128, 512], F32, tag="th", name="th")
        for c in (0, 1):
            chunk = 2 * p + c
            bsl = slice(chunk * 128, (chunk + 1) * 128)
            nc.tensor.matmul(th[:, c * 256:(c + 1) * 256],
                             lhsT=v[:, bsl], rhs=gh, start=True, stop=True)
        # range reduction: k = RNE(theta/2pi) via the 1.5*2^23 magic-add
        # trick (pure fp32 ALU, no dtype conversion), then w = theta - 2pi*k,
        # then a one-period wrap (custom DVE op) handles boundary overshoot
        # and the +pi/2 shift for cos.
        MAGIC = 12582912.0
        kb = wrapp.tile([128, 512], F32, tag="kb", name="kb", bufs=3)
        nc.vector.tensor_scalar(kb, th, 1.0 / (2.0 * PI), MAGIC,
                                op0=OP.mult, op1=OP.add)
        kf = wrapp.tile([128, 512], F32, tag="kf", name="kf", bufs=3)
        nc.scalar.activation(kf, kb, AF.Copy, bias=-MAGIC)
        wr = wrapp.tile([128, 512], F32, tag="wr", name="wr", bufs=3)
        nc.vector.scalar_tensor_tensor(wr, kf, -2.0 * PI, th,
                                       op0=OP.mult, op1=OP.add)
        # wr is already in [-pi-2e-5, pi+2e-5] (k is the RNE quotient), so
        # sin can consume it directly; only the +pi/2-shifted cos path needs
        # the one-period wrap.
        wb = wrapp.tile([128, 512], F32, tag="wb", name="wb", bufs=3)
        nc.vector.add_range_wrap(wb, wr, 0.5 * PI, PI, 2.0 * PI)
        St = csp.tile([128, 512], BF16, tag="St", name="St")
        Ct = csp.tile([128, 512], BF16, tag="Ct", name="Ct")
        nc.scalar.activation(St, wr, AF.Sin, bias=zero_b)
        nc.scalar.activation(Ct, wb, AF.Sin, bias=zero_b)
        for c in (0, 1):
            chunk = 2 * p + c
            side = (chunk // 32) % 2             # 0 -> x1 -> cols 0:256
            first = (chunk % 32) == 0
            last = (chunk % 32) == 31
            co = c * 256
            go = side * 256
            for m in (0, 1):
                lsl = slice(co + m * 128, co + m * 128 + 128)
                nc.tensor.matmul(bankG[m][:, go:go + 256],
                                 lhsT=Ct[:, lsl], rhs=Ct[:, co:co + 256],
                                 start=first, stop=False)
                nc.tensor.matmul(bankG[m][:, go:go + 256],
                                 lhsT=St[:, lsl], rhs=St[:, co:co + 256],
                                 start=False, stop=last)
                nc.tensor.matmul(bankD[m][:, go:go + 256],
                                 lhsT=Ct[:, lsl], rhs=St[:, co:co + 256],
                                 start=first, stop=last)
        if p == N_PACK // 2 - 1:
            emit_epilogue(0)
    emit_epilogue(1)

    es_ps1.close()

    # ---------------- diff + AllReduce (split in two batch-halves) -------
    # handled via emit_epilogue() calls from inside the gram loop; here we
    # only merge the two all-reduced halves.
    grd = []
    drd = []
    for m in (0, 1):
        ga = redp.tile([128, 256], F32, tag=f"ga{m}", name=f"ga{m}")
        nc.sync.dma_start(out=ga, in_=cc_out[0][m * 128:(m + 1) * 128, :])
        gb = redp.tile([128, 256], F32, tag=f"gb{m}", name=f"gb{m}")
        nc.sync.dma_start(out=gb, in_=cc_out[1][m * 128:(m + 1) * 128, :])
        g = redp.tile([128, 256], F32, tag=f"grd{m}", name=f"grd{m}")
        nc.vector.tensor_tensor(g, ga, gb, op=OP.add)
        grd.append(g)
        da = redp.tile([128, 256], F32, tag=f"da{m}", name=f"da{m}")
        nc.sync.dma_start(out=da, in_=cc_out[0][256 + m * 128:256 + (m + 1) * 128, :])
        db = redp.tile([128, 256], F32, tag=f"db{m}", name=f"db{m}")
        nc.sync.dma_start(out=db, in_=cc_out[1][256 + m * 128:256 + (m + 1) * 128, :])
        d = redp.tile([128, 256], F32, tag=f"drd{m}", name=f"drd{m}")
        nc.vector.tensor_tensor(d, da, db, op=OP.add)
        drd.append(d)
    nc.sync.dma_start(out=dbg_d[0], in_=cc_out[0][:])
    nc.sync.dma_start(out=dbg_d[1], in_=cc_out[1][:])

    # ---------------- Hermitianize + scale -> A, X0 ----------------
    es_ps2 = ExitStack()
    tr_ps = es_ps2.enter_context(tc.tile_pool(name="tr_ps", bufs=1, space="PSUM"))
    iterp = es.enter_context(tc.tile_pool(name="iterp", bufs=2))
    af32 = es.enter_context(tc.tile_pool(name="af32", bufs=1))

    # transposes: tb[m] = [Gd^T_m | Dd^T_m]  as [128, 512] psum banks
    tb = [tr_ps.tile([128, 512], F32, tag=f"tb{m}", name=f"tb{m}") for m in (0, 1)]
    for m in (0, 1):
        for nblk in (0, 1):
            msl = slice(m * 128, (m + 1) * 128)
            nc.tensor.transpose(tb[m][:, nblk * 128:(nblk + 1) * 128],
                                in_=grd[nblk][:, msl], identity=ident)
            nc.tensor.transpose(tb[m][:, 256 + nblk * 128:256 + (nblk + 1) * 128],
                                in_=drd[nblk][:, msl], identity=ident)

    Ar = [af32.tile([128, 256], F32, tag=f"Ar{m}", name=f"Ar{m}") for m in (0, 1)]
    Ai = [af32.tile([128, 256], F32, tag=f"Ai{m}", name=f"Ai{m}") for m in (0, 1)]
    Xr = [iterp.tile([128, 256], BF16, tag=f"Xr{m}", name=f"Xr{m}") for m in (0, 1)]
    Xi = [iterp.tile([128, 256], BF16, tag=f"Xi{m}", name=f"Xi{m}") for m in (0, 1)]
    Xn = [iterp.tile([128, 256], BF16, tag=f"Xn{m}", name=f"Xn{m}") for m in (0, 1)]
    for m in (0, 1):
        t = redp.tile([128, 256], F32, tag=f"hz{m}", name=f"hz{m}")
        # A_r = 0.5*alpha*(Gd + Gd^T)
        nc.vector.tensor_tensor(t, grd[m], tb[m][:, 0:256], op=OP.add)
        nc.vector.tensor_scalar(Ar[m], t, 0.5 * ALPHA, None, op0=OP.mult)
        nc.vector.tensor_scalar(Xr[m], t, 0.5 * ALPHA, None, op0=OP.mult)
        t2 = redp.tile([128, 256], F32, tag=f"hz2{m}", name=f"hz2{m}")
        # A_i = alpha*(Dd - Dd^T)
        nc.vector.tensor_tensor(t2, drd[m], tb[m][:, 256:512], op=OP.subtract)
        nc.vector.tensor_scalar(Ai[m], t2, ALPHA, None, op0=OP.mult)
        nc.vector.tensor_scalar(Xi[m], t2, ALPHA, None, op0=OP.mult)
        nc.vector.tensor_scalar(Xn[m], t2, -ALPHA, None, op0=OP.mult)

    if dbg2_d is not None:
        nc.gpsimd.dma_start(out=dbg2_d[0], in_=Xr[0])
        nc.gpsimd.dma_start(out=dbg2_d[1], in_=Xi[0])

    # ---------------- sign iteration ----------------
    es_ps2.close()
    it_ps = es.enter_context(tc.tile_pool(name="it_ps", bufs=1, space="PSUM"))

    def cplx_mm(banks, Lr, Li, Ln, Rr, Ri):
        """banks[m][:, 0:256] = real, [:, 256:512] = imag of L @ R.
        L, R Hermitian-ish: lhsT(real) = L_r (symmetric), lhsT for the
        '-L_i' term = L_i (since L_i^T = -L_i), '+L_i' term = Ln = -L_i."""
        for m in (0, 1):
            orr = banks[m][:, 0:256]
            oii = banks[m][:, 256:512]
            msl = slice(m * 128, (m + 1) * 128)
            nc.tensor.matmul(orr, lhsT=Lr[0][:, msl], rhs=Rr[0],
                             start=True, stop=False)
            nc.tensor.matmul(orr, lhsT=Li[0][:, msl], rhs=Ri[0],
                             start=False, stop=False)
            nc.tensor.matmul(orr, lhsT=Lr[1][:, msl], rhs=Rr[1],
                             start=False, stop=False)
            nc.tensor.matmul(orr, lhsT=Li[1][:, msl], rhs=Ri[1],
                             start=False, stop=True)
            nc.tensor.matmul(oii, lhsT=Lr[0][:, msl], rhs=Ri[0],
                             start=True, stop=False)
            nc.tensor.matmul(oii, lhsT=Ln[0][:, msl], rhs=Rr[0],
                             start=False, stop=False)
            nc.tensor.matmul(oii, lhsT=Lr[1][:, msl], rhs=Ri[1],
                             start=False, stop=False)
            nc.tensor.matmul(oii, lhsT=Ln[1][:, msl], rhs=Rr[1],
                             start=False, stop=True)

    steps = [(a, b, c, False) for (a, b, c) in SCHED]
    steps.append((CUBIC[0], CUBIC[1], 0.0, True))

    for it, (a, b, c, is_last) in enumerate(steps):
        # Y = X^2 (bitwise Hermitian: Gram-of-symmetric products)
        Yb = [it_ps.tile([128, 512], F32, tag=f"pa{m}", name=f"pa{m}") for m in (0, 1)]
        cplx_mm(Yb, Xr, Xi, Xn, Xr, Xi)
        Yr = [iterp.tile([128, 256], BF16, tag=f"Yr{m}", name=f"Yr{m}") for m in (0, 1)]
        Yi = [iterp.tile([128, 256], BF16, tag=f"Yi{m}", name=f"Yi{m}") for m in (0, 1)]
        Yn = [iterp.tile([128, 256], BF16, tag=f"Yn{m}", name=f"Yn{m}") for m in (0, 1)] if not is_last else None
        for m in (0, 1):
            nc.scalar.activation(Yr[m], Yb[m][:, 0:256], AF.Copy)
            nc.scalar.activation(Yi[m], Yb[m][:, 256:512], AF.Copy)
            if not is_last:
                nc.vector.tensor_scalar(Yn[m], Yb[m][:, 256:512], -1.0, None,
                                        op0=OP.mult)
        # V = X*Y (only lhsT = X, which is exactly Hermitian)
        Vb = [it_ps.tile([128, 512], F32, tag=f"pb{m}", name=f"pb{m}") for m in (0, 1)]
        cplx_mm(Vb, Xr, Xi, Xn, Yr, Yi)
        if not is_last:
            Vr = [iterp.tile([128, 256], BF16, tag=f"Vr{m}", name=f"Vr{m}") for m in (0, 1)]
            Vi = [iterp.tile([128, 256], BF16, tag=f"Vi{m}", name=f"Vi{m}") for m in (0, 1)]
            for m in (0, 1):
                nc.scalar.activation(Vr[m], Vb[m][:, 0:256], AF.Copy)
                nc.scalar.activation(Vi[m], Vb[m][:, 256:512], AF.Copy)
            # U = Y*V = X^5 (lhsT = Y, exactly Hermitian; V only as rhs)
            Ub = [it_ps.tile([128, 512], F32, tag=f"pa{m}", name=f"pa{m}") for m in (0, 1)]
            cplx_mm(Ub, Yr, Yi, Yn, Vr, Vi)
            Us = [[None, None], [None, None]]
            for m in (0, 1):
                for comp in (0, 1):
                    src_ = slice(0, 256) if comp == 0 else slice(256, 512)
                    u = wrapp.tile([128, 256], F32, tag=f"us{m}{comp}",
                                   name=f"us{m}{comp}")
                    nc.scalar.activation(u, Ub[m][:, src_], AF.Copy)
                    Us[comp][m] = u

        # t2 = ((c/b)*U + V)*(b/a) + X   (f32, SBUF), per component/Mtile
        t2s = [[None, None], [None, None]]
        for m in (0, 1):
            for comp in (0, 1):
                src_ = slice(0, 256) if comp == 0 else slice(256, 512)
                Xcur = Xr[m] if comp == 0 else Xi[m]
                t2 = wrapp.tile([128, 256], F32, tag=f"cmb{m}{comp}",
                                name=f"cmb{m}{comp}")
                if c != 0.0:
                    t1 = wrapp.tile([128, 256], F32, tag=f"cm1{m}{comp}",
                                    name=f"cm1{m}{comp}")
                    nc.vector.scalar_tensor_tensor(
                        t1, Us[comp][m], c / b, Vb[m][:, src_],
                        op0=OP.mult, op1=OP.add)
                    nc.vector.scalar_tensor_tensor(
                        t2, t1, b / a, Xcur, op0=OP.mult, op1=OP.add)
                else:
                    nc.vector.scalar_tensor_tensor(
                        t2, Vb[m][:, src_], b / a, Xcur,
                        op0=OP.mult, op1=OP.add)
                t2s[comp][m] = t2
        # transpose blocks of t2: tb2[m] = [t2r^T_m | t2i^T_m]
        tb2 = [it_ps.tile([128, 512], F32, tag=f"tb2{m}", name=f"tb2{m}")
               for m in (0, 1)]
        for m in (0, 1):
            msl = slice(m * 128, (m + 1) * 128)
            for nblk in (0, 1):
                nc.tensor.transpose(
                    tb2[m][:, nblk * 128:(nblk + 1) * 128],
                    in_=t2s[0][nblk][:, msl], identity=ident)
                nc.tensor.transpose(
                    tb2[m][:, 256 + nblk * 128:256 + (nblk + 1) * 128],
                    in_=t2s[1][nblk][:, msl], identity=ident)
        # X' = 0.5*a*(t2 + t2^T)  /  0.5*a*(t2 - t2^T)   (exact Hermitian)
        nXr = [iterp.tile([128, 256], BF16, tag=f"Xr{m}", name=f"Xr{m}") for m in (0, 1)]
        nXi = [iterp.tile([128, 256], BF16, tag=f"Xi{m}", name=f"Xi{m}") for m in (0, 1)]
        nXn = [iterp.tile([128, 256], BF16, tag=f"Xn{m}", name=f"Xn{m}") for m in (0, 1)]
        if is_last:
            fXr = [af32.tile([128, 256], F32, tag=f"fXr{m}", name=f"fXr{m}") for m in (0, 1)]
            fXi = [af32.tile([128, 256], F32, tag=f"fXi{m}", name=f"fXi{m}") for m in (0, 1)]
        for m in (0, 1):
            t3r = wrapp.tile([128, 256], F32, tag=f"t3r{m}", name=f"t3r{m}", bufs=1)
            nc.vector.scalar_tensor_tensor(
                t3r, tb2[m][:, 0:256], 1.0, t2s[0][m],
                op0=OP.mult, op1=OP.add)
            t3i = wrapp.tile([128, 256], F32, tag=f"t3i{m}", name=f"t3i{m}", bufs=1)
            nc.vector.scalar_tensor_tensor(
                t3i, tb2[m][:, 256:512], -1.0, t2s[1][m],
                op0=OP.mult, op1=OP.add)
            if is_last:
                nc.vector.tensor_scalar(fXr[m], t3r, 0.5 * a, None, op0=OP.mult)
                nc.vector.tensor_scalar(fXi[m], t3i, 0.5 * a, None, op0=OP.mult)
            else:
                nc.vector.tensor_scalar(nXr[m], t3r, 0.5 * a, None, op0=OP.mult)
                nc.vector.tensor_scalar(nXi[m], t3i, 0.5 * a, None, op0=OP.mult)
                nc.vector.tensor_scalar(nXn[m], t3i, -0.5 * a, None, op0=OP.mult)
        if not is_last:
            Xr, Xi, Xn = nXr, nXi, nXn
            if dbg2_d is not None and it == 0:
                nc.gpsimd.dma_start(out=dbg2_d[2], in_=Xr[0])
                nc.gpsimd.dma_start(out=dbg2_d[3], in_=Xi[0])

    if dbg2_d is not None:
        nc.sync.dma_start(out=dbg2_d[4], in_=fXr[0])
        nc.sync.dma_start(out=dbg2_d[5], in_=fXi[0])

    # ---------------- trace + output ----------------
    partials = []
    for m in (0, 1):
        for comp in (0, 1):
            Xf = fXr[m] if comp == 0 else fXi[m]
            Am = Ar[m] if comp == 0 else Ai[m]
            junk = wrapp.tile([128, 256], F32, tag=f"jk{m}{comp}", name=f"jk{m}{comp}", bufs=1)
            pp = af32.tile([128, 1], F32, tag=f"pp{m}{comp}", name=f"pp{m}{comp}")
            nc.vector.scalar_tensor_tensor(
                junk, Xf, 1.0, Am, op0=OP.mult, op1=OP.mult, accum_out=pp)
            partials.append(pp)
    s1 = af32.tile([128, 1], F32, tag="s1", name="s1")
    nc.vector.tensor_tensor(s1, partials[0], partials[1], op=OP.add)
    s2 = af32.tile([128, 1], F32, tag="s2", name="s2")
    nc.vector.tensor_tensor(s2, partials[2], partials[3], op=OP.add)
    s3 = af32.tile([128, 1], F32, tag="s3", name="s3")
    nc.vector.tensor_tensor(s3, s1, s2, op=OP.add)

    fin_ps = es.enter_context(tc.tile_pool(name="fin_ps", bufs=1, space="PSUM"))
    tr = fin_ps.tile([1, 1], F32)
    nc.tensor.matmul(tr, lhsT=s3, rhs=ones_col, start=True, stop=True)
    outv = af32.tile([1, 1], F32, tag="outv", name="outv")
    nc.scalar.activation(outv, tr, AF.Copy, bias=0.0, scale=-0.5 * S_SCALE)
    nc.sync.dma_start(out=out_d[:], in_=outv)

    es.close()


_CACHED_NC = None


def _get_nc():
    global _CACHED_NC
    if _CACHED_NC is None:
        _CACHED_NC = _build_nc()
    return _CACHED_NC


def _make_in_maps(x1, x0, W1, b1, W2, b2, W3, b3):
    x1 = np.asarray(x1, np.float32)
    x0 = np.asarray(x0, np.float32)
    b3p = np.zeros(10, np.float32)
    b3p[:8] = np.asarray(b3, np.float32)
    biases = np.stack([np.asarray(b1, np.float32),
                       np.asarray(b2, np.float32), b3p], axis=1)
    w1 = _rb(np.asarray(W1, np.float32).T.copy())      # [8, 10]
    w2 = _rb(np.asarray(W2, np.float32).T.copy())      # [10, 10]
    w3 = _rb(np.asarray(W3, np.float32).T.copy())      # [10, 8]
    in_maps = []
    H = B_LOC // 2
    for c in range(N_CORES):
        sl = slice(c * B_LOC, (c + 1) * B_LOC)
        x1s, x0s = x1[sl], x0[sl]
        # chunk order: [x1 half1 | x0 half1 | x1 half2 | x0 half2] so each
        # batch-half yields a complete partial Gram diff for its AllReduce
        xs = np.concatenate([x1s[:H].T, x0s[:H].T, x1s[H:].T, x0s[H:].T],
                            axis=1)   # [8, 16384]
        in_maps.append({
            "xs": np.ascontiguousarray(_rb(xs)),
            "w1": w1, "w2": w2, "w3": w3,
            "biases": np.ascontiguousarray(biases),
        })
    return in_maps


def run(inputs, trace=False):
    nc = _get_nc()
    in_maps = _make_in_maps(**inputs)
    res = run_bass_kernel_spmd(nc, in_maps, core_ids=list(range(N_CORES)),
                               trace=trace)
    val = np.float32(res.results[0]["out"][0, 0])
    return val, res


def kernel(x1, x0, W1, b1, W2, b2, W3, b3) -> np.ndarray:
    val, _ = run(dict(x1=x1, x0=x0, W1=W1, b1=b1, W2=W2, b2=b2,
                      W3=W3, b3=b3))
    return np.asarray(val, dtype=np.float32).reshape(())



# revision 10
# speedup vs baseline: 1.6015x; 1.6015x over previous
"""Trainium2 Bass kernel for nn_DistanceModel1 (quantum-embedding trace
distance model).

Math: psi_b = exp(-i*theta_b)/16, theta = v @ Ghat with v = [h(8), p(7), 1].
With C = cos(theta), S = sin(theta) in [B, 256]:
  256*B*Re(rho) = C^T C + S^T S
  256*B*Im(rho) = C^T S - (C^T S)^T
The answer -0.5*sum|eig(rho1 - rho0)| is the trace norm of the Hermitian
difference, computed with a matrix-sign (polar) iteration: sum|lam| =
tr(sign(A) * A), sign via a tuned odd-quintic schedule + one final cubic.

Implementation notes:
 - MLP is 8x sample-packed: block-diagonal weights on 64/80 partitions so
   each moving column carries 8 samples (PE cost ~1/8 of naive).
 - theta is produced pre-scaled by 1/(2pi); range reduction is a fused
   magic-constant round (one DVE op), fraction on DVE, |fr| on GpSimd,
   and sin/cos as two scalar-engine Sin activations (cos via
   sin(pi/2 - 2pi*|fr|)).
 - Gram matmuls run in fp8 (e4m3) with DoubleRow perf mode: 256-deep
   contraction at N cycles (2x bf16 rate). C/S quantization noise
   averages out over the 64K-sample batch.
 - The two 256x256 Gram-difference matrices are all-reduced in bf16,
   split in two batch-halves so the first collective overlaps the
   second half's compute.
 - Sign iteration: 5 tuned quintic steps + 1 Newton-Schulz cubic,
   bf16 matmuls with fp32 PSUM accumulation; exact Hermitian
   symmetrization each step.

Distribution: data-parallel over batch on 8 NeuronCores, AllReduce of the
Gram partials, then a replicated eigensolve-free trace-norm evaluation.
"""

import numpy as np
import ml_dtypes

import concourse.bass as bass
import concourse.mybir as mybir
import concourse.tile as tile
from concourse import bacc
from concourse.bass_utils import run_bass_kernel_spmd

F32 = mybir.dt.float32
BF16 = mybir.dt.bfloat16
F16 = mybir.dt.float16
F8 = mybir.dt.float8e4

N_CORES = 8
B_TOT = 65536
B_LOC = B_TOT // N_CORES          # 8192 per side per core
BL2 = 2 * B_LOC                   # 16384: per-core samples (x1 + x0)
DIM = 256
N_TILE = 32                       # theta tiles of 512 samples
PI = float(np.pi)
MAGIC = 12582912.0                # 1.5 * 2**23: fp32 round-to-int magic

S_SCALE = 0.0075                  # spectral normalization |lam|max ~ 0.0065
ALPHA = 1.0 / (256.0 * B_TOT * S_SCALE)

# tuned odd-quintic sign-iteration schedule (offline-tuned against the
# actual spectrum; rel trace err ~2.8e-3): x <- a x + b x^3 + c x^5,
# followed by one Newton-Schulz cubic.
SCHED = [
    (5.397828, -15.318763, 11.010532),
    (3.935153, -6.783317, 2.969665),
    (3.872297, -6.812611, 3.133742),
    (3.408851, -5.521410, 2.668857),
    (2.451421, -2.531346, 1.055910),
]
CUBIC = (1.5, -0.5)


def _rh(a):
    return np.asarray(a, np.float16)


def _build_ghat():
    """Ghat [16, 256] scaled by 1/(2pi): th = v @ Ghat gives theta/2pi."""
    n = 8
    d = 256
    bits = (np.arange(d)[:, None] >> (n - 1 - np.arange(n))[None, :]) & 1
    signs = (1.0 - 2.0 * bits).astype(np.float64)           # [256, 8]
    pair = signs[:, :-1] * signs[:, 1:]                      # [256, 7]
    G = np.zeros((16, d), dtype=np.float64)
    for f in range(8):
        col = signs[:, f].copy()
        if f >= 1:
            col += -PI * pair[:, f - 1]
        if f <= 6:
            col += -PI * pair[:, f]
        G[f] = 0.5 * col
    for j in range(7):
        G[8 + j] = 0.5 * pair[:, j]
    G[15] = 0.5 * PI * PI * pair.sum(axis=1)
    return (G / (2.0 * PI)).astype(np.float32)


def _build_nc():
    AF = mybir.ActivationFunctionType
    OP = mybir.AluOpType

    nc = bacc.Bacc(
        "TRN2",
        target_bir_lowering=False,
        debug=False,
        enable_asserts=False,
        num_devices=N_CORES,
    )

    xs_d = nc.dram_tensor("xs", [64, 2048], F16, kind="ExternalInput")
    w1_d = nc.dram_tensor("w1", [64, 80], F16, kind="ExternalInput")
    w2_d = nc.dram_tensor("w2", [80, 80], F16, kind="ExternalInput")
    w3_d = nc.dram_tensor("w3", [80, 64], F16, kind="ExternalInput")
    s8_d = nc.dram_tensor("s8", [64, 64], F16, kind="ExternalInput")
    bias_d = nc.dram_tensor("biases", [80, 3], F32, kind="ExternalInput")
    out_d = nc.dram_tensor("out", [1, 1], F32, kind="ExternalOutput")

    gh_d = nc.inline_tensor(_rh(_build_ghat()), "ghat")          # [16, 256]
    ident_d = nc.inline_tensor(np.eye(128, dtype=np.float32), "ident")
    ones_d = nc.inline_tensor(np.ones((1, BL2), np.float16), "onesrow")

    with tile.TileContext(nc) as tc:
        _body(nc, tc, AF, OP, xs_d, w1_d, w2_d, w3_d, s8_d, bias_d, gh_d,
              ident_d, ones_d, out_d)
    nc.compile()
    return nc


def _body(nc, tc, AF, OP, xs_d, w1_d, w2_d, w3_d, s8_d, bias_d, gh_d,
          ident_d, ones_d, out_d):
    from contextlib import ExitStack
    es = ExitStack()

    constp = es.enter_context(tc.tile_pool(name="constp", bufs=1))

    xs = constp.tile([64, 2048], F16)
    nc.sync.dma_start(out=xs, in_=xs_d[:])
    w1 = constp.tile([64, 80], F16)
    nc.sync.dma_start(out=w1, in_=w1_d[:])
    w2 = constp.tile([80, 80], F16)
    nc.sync.dma_start(out=w2, in_=w2_d[:])
    w3 = constp.tile([80, 64], F16)
    nc.sync.dma_start(out=w3, in_=w3_d[:])
    s8 = constp.tile([64, 64], F16)
    nc.sync.dma_start(out=s8, in_=s8_d[:])
    biases = constp.tile([80, 3], F32)
    nc.sync.dma_start(out=biases, in_=bias_d[:])
    gh = constp.tile([16, 256], F16)
    nc.sync.dma_start(out=gh, in_=gh_d[:])
    ident = constp.tile([128, 128], F32)
    nc.sync.dma_start(out=ident, in_=ident_d[:])
    ones_col = constp.tile([128, 1], F32)
    nc.vector.memset(ones_col, 1.0)
    zero_b = constp.tile([128, 1], F32)
    nc.vector.memset(zero_b, 0.0)
    pio2_b = constp.tile([128, 1], F32)
    nc.vector.memset(pio2_b, 0.5 * PI)

    v = constp.tile([16, BL2], F16)        # [h(0:8); p(8:15); ones(15)]
    nc.sync.dma_start(out=v[15:16, :], in_=ones_d[:])

    # ---------------- MLP (8x sample-packed) ----------------
    es_mlp = ExitStack()
    mlp_ps = es_mlp.enter_context(tc.tile_pool(name="mlp_ps", bufs=2, space="PSUM"))
    actp = es.enter_context(tc.tile_pool(name="actp", bufs=1))

    pm1 = mlp_ps.tile([80, 2048], F32, tag="mp", name="mp")
    for q in range(4):
        sl = slice(q * 512, (q + 1) * 512)
        nc.tensor.matmul(pm1[:, sl], lhsT=w1, rhs=xs[:, sl],
                         start=True, stop=True)
    h1 = actp.tile([80, 2048], F16, tag="h1", name="h1")
    nc.vector.tensor_scalar(h1, pm1, biases[:, 0:1], 0.0,
                            op0=OP.add, op1=OP.max)
    pm2 = mlp_ps.tile([80, 2048], F32, tag="mp", name="mp")
    for q in range(4):
        sl = slice(q * 512, (q + 1) * 512)
        nc.tensor.matmul(pm2[:, sl], lhsT=w2, rhs=h1[:, sl],
                         start=True, stop=True)
    h2 = actp.tile([80, 2048], F16, tag="h2", name="h2")
    nc.scalar.activation(h2, pm2, AF.Relu, bias=biases[:, 1:2])
    pm3 = mlp_ps.tile([80, 2048], F32, tag="mp", name="mp")
    for q in range(4):
        sl = slice(q * 512, (q + 1) * 512)
        nc.tensor.matmul(pm3[0:64, sl], lhsT=w3, rhs=h2[:, sl],
                         start=True, stop=True)
    hfull = actp.tile([64, 2048], F16, tag="hf", name="hf")
    nc.vector.tensor_scalar(hfull, pm3[0:64, :], biases[0:64, 2:3], None,
                            op0=OP.add)
    pm4 = mlp_ps.tile([80, 2048], F32, tag="mp", name="mp")
    for q in range(4):
        sl = slice(q * 512, (q + 1) * 512)
        nc.tensor.matmul(pm4[0:64, sl], lhsT=s8, rhs=hfull[:, sl],
                         start=True, stop=True)
    pc = actp.tile([64, 2048], F16, tag="pc", name="pc")
    nc.vector.tensor_tensor(pc, hfull, pm4[0:64, :], op=OP.mult)
    for g in range(8):
        gs = slice(2048 * g, 2048 * (g + 1))
        nc.sync.dma_start(out=v[0:8, gs], in_=hfull[8 * g:8 * g + 8, :])
        nc.sync.dma_start(out=v[8:15, gs], in_=pc[8 * g:8 * g + 7, :])
    es_mlp.close()

    # ---------------- theta + trig + Gram accumulation ----------------
    es_ps1 = ExitStack()
    th_ps = es_ps1.enter_context(tc.tile_pool(name="th_ps", bufs=2, space="PSUM"))
    gram_ps = es_ps1.enter_context(tc.tile_pool(name="gram_ps", bufs=1, space="PSUM"))
    wrapp = es.enter_context(tc.tile_pool(name="wrapp", bufs=2))
    csp = es.enter_context(tc.tile_pool(name="csp", bufs=2))
    redp = es.enter_context(tc.tile_pool(name="redp", bufs=1))
    dramp = es.enter_context(tc.tile_pool(name="dramp", bufs=1, space="DRAM"))
    cc_in = [dramp.tile([512, 256], BF16, name=f"cc_in{h}") for h in (0, 1)]
    cc_out = [dramp.tile([512, 256], BF16, addr_space="Shared", name=f"cc_out{h}")
              for h in (0, 1)]

    # accumulator banks: [G1_m | G0_m], [D1_m | D0_m] as [128, 512] each
    bankG = [gram_ps.tile([128, 512], F32, tag=f"bg{m}", name=f"bg{m}") for m in (0, 1)]
    bankD = [gram_ps.tile([128, 512], F32, tag=f"bd{m}", name=f"bd{m}") for m in (0, 1)]

    def emit_epilogue(h):
        """extract Gd/Dd = side0 - side1 for batch-half h (bf16), DMA to
        cc_in[h], and kick its AllReduce."""
        for m in (0, 1):
            t1 = redp.tile([128, 256], F32, tag=f"cp{m}{h}", name=f"cp{m}{h}")
            nc.scalar.activation(t1, bankG[m][:, 0:256], AF.Copy)
            gd = redp.tile([128, 256], BF16, tag=f"gd{m}{h}", name=f"gd{m}{h}")
            nc.vector.tensor_tensor(gd, t1, bankG[m][:, 256:512], op=OP.subtract)
            nc.sync.dma_start(out=cc_in[h][m * 128:(m + 1) * 128, :], in_=gd)
            t2 = redp.tile([128, 256], F32, tag=f"cq{m}{h}", name=f"cq{m}{h}")
            nc.scalar.activation(t2, bankD[m][:, 0:256], AF.Copy)
            dd = redp.tile([128, 256], BF16, tag=f"dd{m}{h}", name=f"dd{m}{h}")
            nc.vector.tensor_tensor(dd, t2, bankD[m][:, 256:512], op=OP.subtract)
            nc.sync.dma_start(out=cc_in[h][256 + m * 128:256 + (m + 1) * 128, :],
                              in_=dd)
        nc.gpsimd.collective_compute(
            "AllReduce",
            mybir.AluOpType.add,
            replica_groups=[list(range(N_CORES))],
            ins=[cc_in[h].opt()],
            outs=[cc_out[h].opt()],
        )

    for t in range(N_TILE):
        th = th_ps.tile([128, 4, 256], F32, tag="th", name="th")
        for q in range(4):
            bsl = slice(512 * t + 128 * q, 512 * t + 128 * q + 128)
            nc.tensor.matmul(th[:, q, :], lhsT=v[:, bsl], rhs=gh,
                             start=True, stop=True)
        # k = RNE(th) via fused magic add/sub (fp32 ALU rounds per stage),
        # fr = th - k in [-0.5, 0.5]; sin = Sin(2pi fr); cos via
        # nafr = -|fr| (sign-bit OR, 1-input op on GpSimd) and
        # cos(2pi fr) = Sin(2pi nafr + pi/2) with arg in [-pi/2, pi/2].
        k = wrapp.tile([128, 4, 256], F32, tag="k", name="k")
        nc.vector.tensor_scalar(k, th, MAGIC, -MAGIC, op0=OP.add, op1=OP.add)
        fr = wrapp.tile([128, 4, 256], F32, tag="fr", name="fr")
        nc.vector.scalar_tensor_tensor(fr, k, -1.0, th, op0=OP.mult, op1=OP.add)
        afr = wrapp.tile([128, 4, 256], F32, tag="afr", name="afr")
        nc.scalar.activation(afr, fr, AF.Abs, bias=zero_b, scale=2.0 * PI)
        St = csp.tile([128, 4, 256], F8, tag="St", name="St")
        nc.scalar.activation(St, fr, AF.Sin, bias=zero_b, scale=2.0 * PI)
        Ct = csp.tile([128, 4, 256], F8, tag="Ct", name="Ct")
        nc.scalar.activation(Ct, afr, AF.Sin, bias=pio2_b, scale=-1.0)

        side = (t // 8) % 2                  # 0 -> x1, 1 -> x0
        first = (t % 8) == 0
        last = (t % 8) == 7
        go = side * 256
        DR = mybir.MatmulPerfMode.DoubleRow
        for ks in (0, 2):
            f0 = first and ks == 0
            l0 = last and ks == 2
            for m in (0, 1):
                msl = slice(m * 128, (m + 1) * 128)
                nc.tensor.matmul(bankG[m][:, go:go + 256],
                                 lhsT=Ct[:, ks:ks + 2, msl],
                                 rhs=Ct[:, ks:ks + 2, :],
                                 start=f0, stop=False, perf_mode=DR)
                nc.tensor.matmul(bankG[m][:, go:go + 256],
                                 lhsT=St[:, ks:ks + 2, msl],
                                 rhs=St[:, ks:ks + 2, :],
                                 start=False, stop=l0, perf_mode=DR)
                nc.tensor.matmul(bankD[m][:, go:go + 256],
                                 lhsT=Ct[:, ks:ks + 2, msl],
                                 rhs=St[:, ks:ks + 2, :],
                                 start=f0, stop=l0, perf_mode=DR)
        if t == N_TILE // 2 - 1:
            emit_epilogue(0)
    emit_epilogue(1)

    es_ps1.close()

    # ---------------- merge all-reduced halves ----------------
    grd = []
    drd = []
    for m in (0, 1):
        ga = redp.tile([128, 256], BF16, tag=f"ga{m}", name=f"ga{m}")
        nc.sync.dma_start(out=ga, in_=cc_out[0][m * 128:(m + 1) * 128, :])
        gb = redp.tile([128, 256], BF16, tag=f"gb{m}", name=f"gb{m}")
        nc.sync.dma_start(out=gb, in_=cc_out[1][m * 128:(m + 1) * 128, :])
        g = redp.tile([128, 256], F32, tag=f"grd{m}", name=f"grd{m}")
        nc.vector.tensor_tensor(g, ga, gb, op=OP.add)
        grd.append(g)
        da = redp.tile([128, 256], BF16, tag=f"da{m}", name=f"da{m}")
        nc.sync.dma_start(out=da, in_=cc_out[0][256 + m * 128:256 + (m + 1) * 128, :])
        db = redp.tile([128, 256], BF16, tag=f"db{m}", name=f"db{m}")
        nc.sync.dma_start(out=db, in_=cc_out[1][256 + m * 128:256 + (m + 1) * 128, :])
        d = redp.tile([128, 256], F32, tag=f"drd{m}", name=f"drd{m}")
        nc.vector.tensor_tensor(d, da, db, op=OP.add)
        drd.append(d)

    # ---------------- Hermitianize + scale -> A, X0 ----------------
    es_ps2 = ExitStack()
    tr_ps = es_ps2.enter_context(tc.tile_pool(name="tr_ps", bufs=1, space="PSUM"))
    iterp = es.enter_context(tc.tile_pool(name="iterp", bufs=2))
    af32 = es.enter_context(tc.tile_pool(name="af32", bufs=1))

    # transposes: tb[m] = [Gd^T_m | Dd^T_m]  as [128, 512] psum banks
    tb = [tr_ps.tile([128, 512], F32, tag=f"tb{m}", name=f"tb{m}") for m in (0, 1)]
    for m in (0, 1):
        for nblk in (0, 1):
            msl = slice(m * 128, (m + 1) * 128)
            nc.tensor.transpose(tb[m][:, nblk * 128:(nblk + 1) * 128],
                                in_=grd[nblk][:, msl], identity=ident)
            nc.tensor.transpose(tb[m][:, 256 + nblk * 128:256 + (nblk + 1) * 128],
                                in_=drd[nblk][:, msl], identity=ident)

    Ar = [af32.tile([128, 256], F32, tag=f"Ar{m}", name=f"Ar{m}") for m in (0, 1)]
    Ai = [af32.tile([128, 256], F32, tag=f"Ai{m}", name=f"Ai{m}") for m in (0, 1)]
    Xr = [iterp.tile([128, 256], BF16, tag=f"Xr{m}", name=f"Xr{m}") for m in (0, 1)]
    Xi = [iterp.tile([128, 256], BF16, tag=f"Xi{m}", name=f"Xi{m}") for m in (0, 1)]
    Xn = [iterp.tile([128, 256], BF16, tag=f"Xn{m}", name=f"Xn{m}") for m in (0, 1)]
    for m in (0, 1):
        t = redp.tile([128, 256], F32, tag=f"hz{m}", name=f"hz{m}")
        # A_r = 0.5*alpha*(Gd + Gd^T)
        nc.vector.tensor_tensor(t, grd[m], tb[m][:, 0:256], op=OP.add)
        nc.vector.tensor_scalar(Ar[m], t, 0.5 * ALPHA, None, op0=OP.mult)
        nc.vector.tensor_scalar(Xr[m], t, 0.5 * ALPHA, None, op0=OP.mult)
        t2 = redp.tile([128, 256], F32, tag=f"hz2{m}", name=f"hz2{m}")
        # A_i = alpha*(Dd - Dd^T)
        nc.vector.tensor_tensor(t2, drd[m], tb[m][:, 256:512], op=OP.subtract)
        nc.vector.tensor_scalar(Ai[m], t2, ALPHA, None, op0=OP.mult)
        nc.vector.tensor_scalar(Xi[m], t2, ALPHA, None, op0=OP.mult)
        nc.vector.tensor_scalar(Xn[m], t2, -ALPHA, None, op0=OP.mult)

    # ---------------- sign iteration ----------------
    es_ps2.close()
    it_ps = es.enter_context(tc.tile_pool(name="it_ps", bufs=1, space="PSUM"))

    def cplx_mm(banks, Lr, Li, Ln, Rr, Ri):
        """banks[m][:, 0:256] = real, [:, 256:512] = imag of L @ R.
        L Hermitian: lhsT(real) = L_r (symmetric), lhsT for the
        '-L_i' term = L_i (since L_i^T = -L_i), '+L_i' term = Ln = -L_i."""
        for m in (0, 1):
            orr = banks[m][:, 0:256]
            oii = banks[m][:, 256:512]
            msl = slice(m * 128, (m + 1) * 128)
            nc.tensor.matmul(orr, lhsT=Lr[0][:, msl], rhs=Rr[0],
                             start=True, stop=False)
            nc.tensor.matmul(orr, lhsT=Li[0][:, msl], rhs=Ri[0],
                             start=False, stop=False)
            nc.tensor.matmul(orr, lhsT=Lr[1][:, msl], rhs=Rr[1],
                             start=False, stop=False)
            nc.tensor.matmul(orr, lhsT=Li[1][:, msl], rhs=Ri[1],
                             start=False, stop=True)
            nc.tensor.matmul(oii, lhsT=Lr[0][:, msl], rhs=Ri[0],
                             start=True, stop=False)
            nc.tensor.matmul(oii, lhsT=Ln[0][:, msl], rhs=Rr[0],
                             start=False, stop=False)
            nc.tensor.matmul(oii, lhsT=Lr[1][:, msl], rhs=Ri[1],
                             start=False, stop=False)
            nc.tensor.matmul(oii, lhsT=Ln[1][:, msl], rhs=Rr[1],
                             start=False, stop=True)

    steps = [(a, b, c, False) for (a, b, c) in SCHED]
    steps.append((CUBIC[0], CUBIC[1], 0.0, True))

    for it, (a, b, c, is_last) in enumerate(steps):
        # Y = X^2 (bitwise Hermitian: Gram-of-symmetric products)
        Yb = [it_ps.tile([128, 512], F32, tag=f"pa{m}", name=f"pa{m}") for m in (0, 1)]
        cplx_mm(Yb, Xr, Xi, Xn, Xr, Xi)
        Yr = [iterp.tile([128, 256], BF16, tag=f"Yr{m}", name=f"Yr{m}") for m in (0, 1)]
        Yi = [iterp.tile([128, 256], BF16, tag=f"Yi{m}", name=f"Yi{m}") for m in (0, 1)]
        Yn = [iterp.tile([128, 256], BF16, tag=f"Yn{m}", name=f"Yn{m}") for m in (0, 1)] if not is_last else None
        for m in (0, 1):
            nc.scalar.activation(Yr[m], Yb[m][:, 0:256], AF.Copy)
            nc.scalar.activation(Yi[m], Yb[m][:, 256:512], AF.Copy)
            if not is_last:
                nc.vector.tensor_scalar(Yn[m], Yb[m][:, 256:512], -1.0, None,
                                        op0=OP.mult)
        # V = X*Y (only lhsT = X, which is exactly Hermitian)
        Vb = [it_ps.tile([128, 512], F32, tag=f"pb{m}", name=f"pb{m}") for m in (0, 1)]
        cplx_mm(Vb, Xr, Xi, Xn, Yr, Yi)
        if not is_last:
            Vr = [iterp.tile([128, 256], BF16, tag=f"Vr{m}", name=f"Vr{m}") for m in (0, 1)]
            Vi = [iterp.tile([128, 256], BF16, tag=f"Vi{m}", name=f"Vi{m}") for m in (0, 1)]
            for m in (0, 1):
                nc.scalar.activation(Vr[m], Vb[m][:, 0:256], AF.Copy)
                nc.scalar.activation(Vi[m], Vb[m][:, 256:512], AF.Copy)
            # U = Y*V = X^5 (lhsT = Y, exactly Hermitian; V only as rhs)
            Ub = [it_ps.tile([128, 512], F32, tag=f"pa{m}", name=f"pa{m}") for m in (0, 1)]
            cplx_mm(Ub, Yr, Yi, Yn, Vr, Vi)
            Us = [[None, None], [None, None]]
            for m in (0, 1):
                for comp in (0, 1):
                    src_ = slice(0, 256) if comp == 0 else slice(256, 512)
                    u = wrapp.tile([128, 256], F32, tag=f"us{m}{comp}",
                                   name=f"us{m}{comp}")
                    nc.scalar.activation(u, Ub[m][:, src_], AF.Copy)
                    Us[comp][m] = u

        # t2 = ((c/b)*U + V)*(b/a) + X   (f32, SBUF), per component/Mtile
        t2s = [[None, None], [None, None]]
        for m in (0, 1):
            for comp in (0, 1):
                src_ = slice(0, 256) if comp == 0 else slice(256, 512)
                Xcur = Xr[m] if comp == 0 else Xi[m]
                t2 = wrapp.tile([128, 256], F32, tag=f"cmb{m}{comp}",
                                name=f"cmb{m}{comp}")
                if c != 0.0:
                    t1 = wrapp.tile([128, 256], F32, tag=f"cm1{m}{comp}",
                                    name=f"cm1{m}{comp}")
                    nc.vector.scalar_tensor_tensor(
                        t1, Us[comp][m], c / b, Vb[m][:, src_],
                        op0=OP.mult, op1=OP.add)
                    nc.vector.scalar_tensor_tensor(
                        t2, t1, b / a, Xcur, op0=OP.mult, op1=OP.add)
                else:
                    nc.vector.scalar_tensor_tensor(
                        t2, Vb[m][:, src_], b / a, Xcur,
                        op0=OP.mult, op1=OP.add)
                t2s[comp][m] = t2
        # transpose blocks of t2: tb2[m] = [t2r^T_m | t2i^T_m]
        tb2 = [it_ps.tile([128, 512], F32, tag=f"tb2{m}", name=f"tb2{m}")
               for m in (0, 1)]
        for m in (0, 1):
            msl = slice(m * 128, (m + 1) * 128)
            for nblk in (0, 1):
                nc.tensor.transpose(
                    tb2[m][:, nblk * 128:(nblk + 1) * 128],
                    in_=t2s[0][nblk][:, msl], identity=ident)
                nc.tensor.transpose(
                    tb2[m][:, 256 + nblk * 128:256 + (nblk + 1) * 128],
                    in_=t2s[1][nblk][:, msl], identity=ident)
        # X' = 0.5*a*(t2 + t2^T)  /  0.5*a*(t2 - t2^T)   (exact Hermitian)
        nXr = [iterp.tile([128, 256], BF16, tag=f"Xr{m}", name=f"Xr{m}") for m in (0, 1)]
        nXi = [iterp.tile([128, 256], BF16, tag=f"Xi{m}", name=f"Xi{m}") for m in (0, 1)]
        nXn = [iterp.tile([128, 256], BF16, tag=f"Xn{m}", name=f"Xn{m}") for m in (0, 1)]
        if is_last:
            fXr = [af32.tile([128, 256], F32, tag=f"fXr{m}", name=f"fXr{m}") for m in (0, 1)]
            fXi = [af32.tile([128, 256], F32, tag=f"fXi{m}", name=f"fXi{m}") for m in (0, 1)]
        for m in (0, 1):
            t3r = wrapp.tile([128, 256], F32, tag=f"t3r{m}", name=f"t3r{m}", bufs=1)
            nc.vector.scalar_tensor_tensor(
                t3r, tb2[m][:, 0:256], 1.0, t2s[0][m],
                op0=OP.mult, op1=OP.add)
            t3i = wrapp.tile([128, 256], F32, tag=f"t3i{m}", name=f"t3i{m}", bufs=1)
            nc.vector.scalar_tensor_tensor(
                t3i, tb2[m][:, 256:512], -1.0, t2s[1][m],
                op0=OP.mult, op1=OP.add)
            if is_last:
                nc.vector.tensor_scalar(fXr[m], t3r, 0.5 * a, None, op0=OP.mult)
                nc.vector.tensor_scalar(fXi[m], t3i, 0.5 * a, None, op0=OP.mult)
            else:
                nc.vector.tensor_scalar(nXr[m], t3r, 0.5 * a, None, op0=OP.mult)
                nc.vector.tensor_scalar(nXi[m], t3i, 0.5 * a, None, op0=OP.mult)
                nc.vector.tensor_scalar(nXn[m], t3i, -0.5 * a, None, op0=OP.mult)
        if not is_last:
            Xr, Xi, Xn = nXr, nXi, nXn

    # ---------------- trace + output ----------------
    partials = []
    for m in (0, 1):
        for comp in (0, 1):
            Xf = fXr[m] if comp == 0 else fXi[m]
            Am = Ar[m] if comp == 0 else Ai[m]
            junk = wrapp.tile([128, 256], F32, tag=f"jk{m}{comp}", name=f"jk{m}{comp}", bufs=1)
            pp = af32.tile([128, 1], F32, tag=f"pp{m}{comp}", name=f"pp{m}{comp}")
            nc.vector.scalar_tensor_tensor(
                junk, Xf, 1.0, Am, op0=OP.mult, op1=OP.mult, accum_out=pp)
            partials.append(pp)
    s1 = af32.tile([128, 1], F32, tag="s1", name="s1")
    nc.vector.tensor_tensor(s1, partials[0], partials[1], op=OP.add)
    s2 = af32.tile([128, 1], F32, tag="s2", name="s2")
    nc.vector.tensor_tensor(s2, partials[2], partials[3], op=OP.add)
    s3 = af32.tile([128, 1], F32, tag="s3", name="s3")
    nc.vector.tensor_tensor(s3, s1, s2, op=OP.add)

    fin_ps = es.enter_context(tc.tile_pool(name="fin_ps", bufs=1, space="PSUM"))
    tr = fin_ps.tile([1, 1], F32)
    nc.tensor.matmul(tr, lhsT=s3, rhs=ones_col, start=True, stop=True)
    outv = af32.tile([1, 1], F32, tag="outv", name="outv")
    nc.scalar.activation(outv, tr, AF.Copy, bias=0.0, scale=-0.5 * S_SCALE)
    nc.sync.dma_start(out=out_d[:], in_=outv)

    es.close()


_CACHED_NC = None


def _get_nc():
    global _CACHED_NC
    if _CACHED_NC is None:
        _CACHED_NC = _build_nc()
    return _CACHED_NC


def _blockdiag8(w):
    r, c = w.shape
    out = np.zeros((8 * r, 8 * c), dtype=np.float32)
    for g in range(8):
        out[g * r:(g + 1) * r, g * c:(g + 1) * c] = w
    return out


def _make_in_maps(x1, x0, W1, b1, W2, b2, W3, b3):
    x1 = np.asarray(x1, np.float32)
    x0 = np.asarray(x0, np.float32)
    w1 = _rh(_blockdiag8(np.asarray(W1, np.float32).T))    # [64, 80]
    w2 = _rh(_blockdiag8(np.asarray(W2, np.float32).T))    # [80, 80]
    w3 = _rh(_blockdiag8(np.asarray(W3, np.float32).T))    # [80, 64]
    s8 = np.zeros((64, 64), np.float32)
    for m in range(64):
        if m % 8 != 7:
            s8[m + 1, m] = 1.0
    s8 = _rh(s8)
    biases = np.zeros((80, 3), np.float32)
    biases[:, 0] = np.tile(np.asarray(b1, np.float32), 8)
    biases[:, 1] = np.tile(np.asarray(b2, np.float32), 8)
    biases[0:64, 2] = np.tile(np.asarray(b3, np.float32), 8)
    in_maps = []
    H = B_LOC // 2
    for c in range(N_CORES):
        sl = slice(c * B_LOC, (c + 1) * B_LOC)
        x1s, x0s = x1[sl], x0[sl]
        # sample order: [x1 half1 | x0 half1 | x1 half2 | x0 half2] so each
        # batch-half yields a complete partial Gram diff for its AllReduce
        xo = np.concatenate([x1s[:H], x0s[:H], x1s[H:], x0s[H:]], axis=0)
        # 8x packing: partition block g holds features of samples
        # [2048g, 2048(g+1)); column t = sample 2048g + t.
        xp = np.ascontiguousarray(
            xo.reshape(8, 2048, 8).transpose(0, 2, 1).reshape(64, 2048))
        in_maps.append({
            "xs": _rh(xp),
            "w1": w1, "w2": w2, "w3": w3, "s8": s8,
            "biases": np.ascontiguousarray(biases),
        })
    return in_maps


def run(inputs, trace=False):
    nc = _get_nc()
    in_maps = _make_in_maps(**inputs)
    res = run_bass_kernel_spmd(nc, in_maps, core_ids=list(range(N_CORES)),
                               trace=trace)
    val = np.float32(res.results[0]["out"][0, 0])
    return val, res


def kernel(x1, x0, W1, b1, W2, b2, W3, b3) -> np.ndarray:
    val, _ = run(dict(x1=x1, x0=x0, W1=W1, b1=b1, W2=W2, b2=b2,
                      W3=W3, b3=b3))
    return np.asarray(val, dtype=np.float32).reshape(())
